# revision 8
# baseline (speedup 1.0000x reference)
"""GCNN (batched SpMM + GEMM + bias + ReLU) Trainium2 kernel.

Strategy: dense block-streamed SpMM (no gather, no per-edge DMA
descriptors).

Per-core work (one graph per NeuronCore, 8 graphs / 8 cores):
  phase 0: y = x @ W            (bf16 PE matmuls, y tiles stay in SBUF)
  phase 1: out^T = A^T-blocks streamed dense:
      out^T[ch, dest] = sum_k y_k^T @ A^T[k-block, dest-block]
    - A^T built dense on host, bf16, in a pass-major layout
      [128 src-lane, pass | k | dest-col] so each DMA call moves
      KCHUNK k-blocks with one large contiguous descriptor per
      partition at full HBM bandwidth
    - the last src block (16 real rows of 128) is stored and streamed
      as a [16, span] slab - its 112 zero partitions are never moved
    - y_k (bf16, SBUF-resident) is the matmul *stationary* operand;
      A^T slabs are the *moving* operand
    - PSUM accumulates out^T per dest pass over all 79 k-blocks;
      7 dest passes of 12 groups (3 PSUM banks each) ping-pong across
      6 PSUM banks so evictions overlap the next pass's matmuls
    - eviction: single ACT op relu(psum + bias) (bias is per-partition
      in the out^T orientation) to bf16, DMA out; host transposes and
      upcasts

The SWDGE dma_gather approach (one descriptor per edge) is descriptor-
generation-bound on the GPSIMD engine (~6 ns/edge = 2.2 ms/core); the
dense stream moves more bytes (~200 MB vs ~115 MB) but at full DMA
bandwidth with zero GPSIMD work and pure PE streaming.

SPMD: one NEFF for all 8 cores; per-core tensors differ only in data.
"""

import sys

if "/opt/trn_rl_repo" not in sys.path:
    sys.path.insert(0, "/opt/trn_rl_repo")

import numpy as np
import ml_dtypes

import concourse.bacc as bacc
import concourse.mybir as mybir
from concourse import tile
from concourse.bass_utils import run_bass_kernel_spmd

BF16 = ml_dtypes.bfloat16

C = 128            # channels (C_IN == C_OUT == 128)
N = 10000          # nodes per graph
NB = (N + 127) // 128          # 79 src blocks
KLAST = NB - 1                 # last src block: only 16 real rows
KLAST_ROWS = N - KLAST * 128   # 16
GROUPS_PER_PASS = 12           # 3 PSUM banks per pass, 6 banks ping-pong
BANK_COLS = 512                # fp32 columns per PSUM bank
KCHUNK = 8                     # k-blocks per DMA slab


def _passes():
    out = []
    g = 0
    while g * 128 < N:
        c0 = g * 128
        c1 = min((g + GROUPS_PER_PASS) * 128, N)
        out.append((c0, c1 - c0))          # (first dest col, real span)
        g += GROUPS_PER_PASS
    return out


PASSES = _passes()                          # [(col0, span), ...]
# full-height slab region: k in [0, KLAST); short slab (16 rows) for KLAST
PASS_OFF = np.concatenate(
    [[0], np.cumsum([KLAST * s for _, s in PASSES])]).astype(np.int64)
AT_COLS = int(PASS_OFF[-1])                 # 78 * 10000 = 780000
# the short k=78 slab lives in a separate [16, N] tensor, pass-major too
PASS_OFF2 = np.concatenate(
    [[0], np.cumsum([s for _, s in PASSES])]).astype(np.int64)


# ---------------------------------------------------------------- host prep

def prep_in_maps(x, edge_rows, edge_cols, edge_vals, W, b):
    """Build per-core input maps: xT (bf16), W (bf16), bT (f32 bias in
    out^T orientation), AT / AT2 (dense A^T, bf16, pass-major)."""
    x = np.asarray(x)
    W16 = np.asarray(W, dtype=np.float32).astype(BF16)
    bT = np.ascontiguousarray(
        np.asarray(b, dtype=np.float32)[:, None] * np.ones((1, 1), np.float32))

    spans = np.array([s for _, s in PASSES], dtype=np.int64)

    in_maps = []
    for g in range(x.shape[0]):
        rows = np.asarray(edge_rows[g], dtype=np.int64)
        cols = np.asarray(edge_cols[g], dtype=np.int64)
        vals = np.asarray(edge_vals[g], dtype=np.float32)
        k = cols // 128
        c_loc = cols % 128
        p = rows // (GROUPS_PER_PASS * 128)
        r_rel = rows - p * GROUPS_PER_PASS * 128

        at = np.zeros(C * AT_COLS, dtype=np.float32)
        m = k < KLAST
        idx = (c_loc[m] * AT_COLS
               + PASS_OFF[p[m]] + k[m] * spans[p[m]] + r_rel[m])
        np.add.at(at, idx, vals[m])

        at2 = np.zeros(KLAST_ROWS * N, dtype=np.float32)
        m2 = ~m
        idx2 = c_loc[m2] * N + PASS_OFF2[p[m2]] + r_rel[m2]
        np.add.at(at2, idx2, vals[m2])

        in_maps.append({
            "xT": np.ascontiguousarray(x[g].T.astype(BF16)),
            "W": W16,
            "bT": bT,
            "AT": at.reshape(C, AT_COLS).astype(BF16),
            "AT2": at2.reshape(KLAST_ROWS, N).astype(BF16),
        })
    return in_maps


# ---------------------------------------------------------------- device IR

def build_nc():
    f32 = mybir.dt.float32
    bf16 = mybir.dt.bfloat16

    nc = bacc.Bacc("TRN2")
    xT_d = nc.dram_tensor("xT", [C, N], bf16, kind="ExternalInput")
    W_d = nc.dram_tensor("W", [C, C], bf16, kind="ExternalInput")
    bT_d = nc.dram_tensor("bT", [C, 1], f32, kind="ExternalInput")
    AT_d = nc.dram_tensor("AT", [C, AT_COLS], bf16, kind="ExternalInput")
    AT2_d = nc.dram_tensor("AT2", [KLAST_ROWS, N], bf16, kind="ExternalInput")
    outT_d = nc.dram_tensor("outT", [C, N], bf16, kind="ExternalOutput")

    max_slab = KCHUNK * GROUPS_PER_PASS * 128   # bf16 elems per partition

    with tile.TileContext(nc) as tc:
        with (
            tc.tile_pool(name="const", bufs=1) as constp,
            tc.tile_pool(name="ypool", bufs=NB) as ypool,
            tc.tile_pool(name="p0ps", bufs=2, space="PSUM") as p0ps,
            tc.tile_pool(name="atp", bufs=4) as atp,
            tc.tile_pool(name="at2p", bufs=2) as at2p,
            tc.tile_pool(name="acc", bufs=6, space="PSUM") as accp,
            tc.tile_pool(name="ev", bufs=3) as evp,
        ):
            # ---- constants
            w_t = constp.tile([C, C], bf16, tag="w")
            nc.sync.dma_start(out=w_t[:], in_=W_d[:])
            bias_t = constp.tile([C, 1], f32, tag="bias")
            nc.sync.dma_start(out=bias_t[:], in_=bT_d[:])
            x_t = constp.tile([C, N], bf16, tag="x")
            nc.scalar.dma_start(out=x_t[:], in_=xT_d[:])

            # ---- phase 0: y = x @ W, tiles kept resident in SBUF (bf16)
            y_tiles = []
            for t in range(NB):
                rows = min(128, N - t * 128)
                yps = p0ps.tile([128, C], f32, tag="yps")
                nc.tensor.matmul(yps[:rows, :],
                                 x_t[:, t * 128:t * 128 + rows],
                                 w_t[:], start=True, stop=True)
                ysb = ypool.tile([128, C], bf16, tag="y", name=f"y_{t}")
                nc.vector.tensor_copy(ysb[:rows, :], yps[:rows, :])
                y_tiles.append(ysb)

            # ---- phase 1: stream A^T slabs, accumulate out^T in PSUM
            dma_engines = [nc.sync, nc.scalar]
            slab_i = 0
            for pi, (col0, span) in enumerate(PASSES):
                nbank = (span + BANK_COLS - 1) // BANK_COLS
                ps = []
                for bi in range(nbank):
                    pt = accp.tile([128, BANK_COLS], f32, tag="acc",
                                   name=f"acc_{col0}_{bi}")
                    ps.append(pt)
                for k0 in range(0, KLAST, KCHUNK):
                    kn = min(KCHUNK, KLAST - k0)
                    at_t = atp.tile([128, max_slab], bf16, tag="at")
                    lo = int(PASS_OFF[pi]) + k0 * span
                    dma_engines[slab_i % 2].dma_start(
                        out=at_t[:, :kn * span],
                        in_=AT_d[:, lo:lo + kn * span])
                    slab_i += 1
                    for kk in range(kn):
                        k = k0 + kk
                        for bi in range(nbank):
                            ncols = min(BANK_COLS, span - bi * BANK_COLS)
                            off = kk * span + bi * BANK_COLS
                            nc.tensor.matmul(
                                ps[bi][:, :ncols],
                                y_tiles[k][:],
                                at_t[:, off:off + ncols],
                                start=(k == 0), stop=False,
                                skip_group_check=True,
                            )
                # short k=78 slab: only the 16 real src rows
                at2_t = at2p.tile([KLAST_ROWS, GROUPS_PER_PASS * 128], bf16,
                                  tag="at2")
                lo2 = int(PASS_OFF2[pi])
                nc.scalar.dma_start(out=at2_t[:, :span],
                                    in_=AT2_d[:, lo2:lo2 + span])
                for bi in range(nbank):
                    ncols = min(BANK_COLS, span - bi * BANK_COLS)
                    nc.tensor.matmul(
                        ps[bi][:, :ncols],
                        y_tiles[KLAST][:KLAST_ROWS, :],
                        at2_t[:, bi * BANK_COLS:bi * BANK_COLS + ncols],
                        start=False, stop=True,
                        skip_group_check=True,
                    )
                # evict: relu(psum + bias) in one ACT op to bf16, DMA out
                for bi in range(nbank):
                    c0 = col0 + bi * BANK_COLS
                    real = min(BANK_COLS, col0 + span - c0)
                    ot = evp.tile([128, BANK_COLS], bf16, tag="ot")
                    nc.scalar.activation(
                        out=ot[:, :real], in_=ps[bi][:, :real],
                        func=mybir.ActivationFunctionType.Relu,
                        bias=bias_t[:, 0:1])
                    nc.sync.dma_start(out=outT_d[:, c0:c0 + real],
                                      in_=ot[:, :real])

    nc.finalize()
    return nc


# ---------------------------------------------------------------- entry

def kernel(x, edge_rows, edge_cols, edge_vals, W, b):
    x = np.asarray(x)
    in_maps = prep_in_maps(x, edge_rows, edge_cols, edge_vals, W, b)
    nc = build_nc()
    res = run_bass_kernel_spmd(nc, in_maps, list(range(x.shape[0])))
    out = np.stack([np.asarray(r["outT"]).astype(np.float32).T
                    for r in res.results])
    return out
